# revision 5
# baseline (speedup 1.0000x reference)
"""BGE-M3 sparse-embedding head (matvec + relu + scatter-max into (B, V))
as a Bass/Tile kernel on 8 Trainium2 NeuronCores.

Sharding: data-parallel over batch; each core computes 4 of 32 rows.

Per core / per row (1024 tokens):
  1. matvec on the PE: host uploads hidden TRANSPOSED ([h, token-slot] fp16);
     lhsT = w-block replicated across 128 columns, rhs = x^T -> every psum
     partition holds the full tw vector. Row 0 is copied to SBUF (Act) and
     round-tripped through a DRAM scratch so a strided DMA gather lands tw in
     token-major [128, 8] layout.
  2. Each token is host-assigned to (band-chunk k, slot s): chunk k holds only
     tokens whose dense column f = v % 1954 lies in fixed band k (~245 cols).
     DVE builds, per chunk, rk[slot, 0:246|246:262] = onehot(off)*tw covering
     both the band columns and 16 fixup member slots, plus one fused route
     one-hot akh[slot, k, p] (p = v // 1954, or fixup class q, or -1).
  3. PE assembles the dense [128, 1954] row in PSUM with one ~245-col matmul
     per band (disjoint -> start=stop=True, no zeroing pass) + one 16-col
     fixup matmul per chunk sharing the same lhsT. Act casts PSUM -> fp16.
  4. Duplicate ids / band-overflow tokens go to <=128 fixup classes: members
     accumulate into psum cols [1954, 1970), DVE reduce_max -> per-class value,
     scattered by ONE 128-index indirect DMA after the dense row write (both
     on the gpsimd SWDGE queue, so ordering is guaranteed by the queue).
Output is written fp16 (250112-padded) and upcast to f32 on the host.
"""

import numpy as np

import concourse.bass as bass
import concourse.mybir as mybir
import concourse.tile as tile
from concourse.bass import IndirectOffsetOnAxis
from concourse.bass_utils import run_bass_kernel_spmd

V = 250002
NCORES = 8
B, L, H = 32, 1024, 1024
BS = B // NCORES            # batch rows per core (4)
P = 128
W = 1954                    # dense row width per partition (128*1954 >= V)
VPAD = P * W                # 250112
NB = 8                      # h-blocks / band-chunks per row
MAXCLS = P                  # fixup classes per row
MAXMEM = 8                  # member slots per (fixup class, chunk)
FIXC0 = 246                 # rk column where fixup member slots start
MEMMARK = 2000.0            # iwc value marking member slot m: MEMMARK + m
RKW = FIXC0 + MAXMEM        # 254
F32 = mybir.dt.float32
F16 = mybir.dt.float16
I32 = mybir.dt.int32

# fixed band boundaries over [0, W)
BB = [round(W * k / NB) for k in range(NB + 1)]   # [0,244,489,733,977,1221,1466,1710,1954]
PSUM_BANK = 512  # f32 elements per PSUM bank; matmul out must not cross banks


def _band_segments():
    """Per band k: list of (lo, hi) psum column ranges split at bank bounds."""
    segs = []
    for k in range(NB):
        lo, hi = BB[k], BB[k + 1]
        cuts = [c for c in range(PSUM_BANK, 4 * PSUM_BANK, PSUM_BANK) if lo < c < hi]
        pts = [lo] + cuts + [hi]
        segs.append([(pts[i], pts[i + 1]) for i in range(len(pts) - 1)])
    return segs


SEGS = _band_segments()

_MAX_WAITS = 1


def _split_excess_waits(nc, cap=_MAX_WAITS):
    """walrus's gen3 codegen rejects >1 sync-wait per instruction; move the
    excess onto NoOps inserted just before (same engine => order kept)."""
    n = 0
    for func in nc.m.functions:
        for bb in func.blocks:
            newlist = []
            for ins in bb.instructions:
                si = getattr(ins, "sync_info", None)
                if si is not None and si.on_wait and len(si.on_wait) > cap:
                    waits = list(si.on_wait)
                    extra, keep = waits[:-cap], waits[-cap:]
                    while extra:
                        chunk, extra = extra[:cap], extra[cap:]
                        nop = mybir.InstNoOp(
                            name=f"{ins.name}-wsplit-{n}", ins=[], outs=[]
                        )
                        nop.engine = ins.engine
                        nop.sync_info = mybir.SyncInfo(on_wait=chunk, on_update=[])
                        newlist.append(nop)
                        n += 1
                    ins.sync_info = mybir.SyncInfo(
                        on_wait=keep, on_update=list(si.on_update)
                    )
                newlist.append(ins)
            bb.instructions = newlist
    return n


def _build_program():
    nc = bass.Bass()
    Op = mybir.AluOpType

    xt2 = nc.declare_dram_parameter("xt2", [BS * P, NB * L], F16, isOutput=False)
    wrep = nc.declare_dram_parameter("wrep", [P, NB * P], F16, isOutput=False)
    ipt = nc.declare_dram_parameter("ipt", [P, NB * P], F16, isOutput=False)
    iwc = nc.declare_dram_parameter("iwc", [P, RKW], F32, isOutput=False)
    bcol = nc.declare_dram_parameter("bcol", [P, 1], F32, isOutput=False)
    route = nc.declare_dram_parameter("route", [P, BS * NB], F16, isOutput=False)
    offrel = nc.declare_dram_parameter("offrel", [P, BS * NB], F32, isOutput=False)
    fixgid = nc.declare_dram_parameter("fixgid", [P, BS], I32, isOutput=False)
    outs = [
        nc.declare_dram_parameter(f"out{r}", [VPAD], F16, isOutput=True)
        for r in range(BS)
    ]
    twd = nc.dram_tensor("twd", [BS, L], F16, kind="Internal")

    with tile.TileContext(nc) as tc:
        with (
            tc.tile_pool(name="pers", bufs=1) as pers,
            tc.tile_pool(name="xt", bufs=3) as xt_tp,
            tc.tile_pool(name="akh", bufs=2) as akh_tp,
            tc.tile_pool(name="rk", bufs=3) as rk_tp,
            tc.tile_pool(name="tw", bufs=2) as tw_tp,
            tc.tile_pool(name="twrow", bufs=2) as twrow_tp,
            tc.tile_pool(name="dense", bufs=2) as dense_tp,
            tc.tile_pool(name="fixv", bufs=2) as fixv_tp,
            tc.tile_pool(name="ptw", bufs=2, space="PSUM") as ptw_tp,
            tc.tile_pool(name="psd", bufs=1, space="PSUM") as psd_tp,
        ):
            # ---- wrep first (matvec(0) needs it), then prefetch 2 rows of input ----
            wrep_t = pers.tile([P, NB * P], F16, tag="wrep")
            nc.sync.dma_start(out=wrep_t[:], in_=wrep[:])

            xt_tiles = {}

            def emit_in(r):
                xt_t = xt_tp.tile([P, NB * L], F16, tag="xt")
                nc.sync.dma_start(
                    out=xt_t[:, 0 : 4 * L],
                    in_=xt2[r * P : (r + 1) * P, 0 : 4 * L],
                )
                nc.scalar.dma_start(
                    out=xt_t[:, 4 * L : 8 * L],
                    in_=xt2[r * P : (r + 1) * P, 4 * L : 8 * L],
                )
                xt_tiles[r] = xt_t

            emit_in(0)
            emit_in(1)

            # ---- remaining constants (needed ~10us in, after the prefetches) ----
            ipt_t = pers.tile([P, NB * P], F16, tag="ipt")
            nc.scalar.dma_start(out=ipt_t[:], in_=ipt[:])
            iwc_t = pers.tile([P, RKW], F32, tag="iwc")
            nc.scalar.dma_start(out=iwc_t[:], in_=iwc[:])
            bcol_t = pers.tile([P, 1], F32, tag="bcol")
            nc.scalar.dma_start(out=bcol_t[:], in_=bcol[:])
            route_t = pers.tile([P, BS * NB], F16, tag="route")
            nc.scalar.dma_start(out=route_t[:], in_=route[:])
            offr_t = pers.tile([P, BS * NB], F32, tag="offr")
            nc.scalar.dma_start(out=offr_t[:], in_=offrel[:])
            fg_t = pers.tile([P, BS], I32, tag="fg")
            nc.scalar.dma_start(out=fg_t[:], in_=fixgid[:])

            ptw_tiles = {}

            def emit_matvec(r):
                ptw = ptw_tp.tile([P, L], F32, tag="ptw")
                xt_t = xt_tiles[r]
                for b in range(NB):
                    for hh in range(2):
                        nc.tensor.matmul(
                            out=ptw[:, hh * 512 : (hh + 1) * 512],
                            lhsT=wrep_t[:, b * P : (b + 1) * P],
                            rhs=xt_t[:, b * L + hh * 512 : b * L + (hh + 1) * 512],
                            start=(b == 0),
                            stop=(b == NB - 1),
                        )
                ptw_tiles[r] = ptw

            emit_matvec(0)

            for r in range(BS):
                c0 = r * NB
                if r + 2 < BS:
                    emit_in(r + 2)
                if r + 1 < BS:
                    emit_matvec(r + 1)  # PE fills the tw round-trip latency
                ptw = ptw_tiles.pop(r)
                xt_tiles.pop(r)
                # ---- tw extraction: psum row0 -> SBUF (transposed) -> DRAM -> [128, 8] ----
                twrow = twrow_tp.tile([1, L], F16, tag="twrow")
                nc.scalar.copy(
                    out=twrow[:].rearrange("o (p j) -> o j p", j=NB),
                    in_=ptw[0:1, :].rearrange("o (j p) -> o j p", p=P),
                )
                nc.scalar.dma_start(out=twd[r : r + 1, :], in_=twrow[:])
                twg = tw_tp.tile([P, NB], F16, tag="twg")
                nc.scalar.dma_start(
                    out=twg[:].rearrange("p (j o) -> p j o", o=1),
                    in_=twd[r : r + 1, :].rearrange("o (p j) -> p j o", p=P),
                )
                # relu(tw + b)
                tw_t = tw_tp.tile([P, NB], F32, tag="tw")
                nc.vector.tensor_scalar(
                    out=tw_t[:], in0=twg[:],
                    scalar1=bcol_t[:, 0:1], scalar2=0.0,
                    op0=Op.add, op1=Op.max,
                )
                # ---- route one-hot for all 8 chunks in one op ----
                akh = akh_tp.tile([P, NB * P], F16, tag="akh")
                nc.vector.tensor_tensor(
                    out=akh[:].rearrange("p (k q) -> p k q", q=P),
                    in0=ipt_t[:].rearrange("p (k q) -> p k q", q=P),
                    in1=route_t[:, c0 : c0 + NB].unsqueeze(2).broadcast_to((P, NB, P)),
                    op=Op.is_equal,
                )
                # ---- per-chunk rk + band/fixup matmuls ----
                psd = psd_tp.tile([P, 4 * PSUM_BANK], F32, tag="psd")
                for k in range(NB):
                    rk = rk_tp.tile([P, RKW], F16, tag="rk")
                    nc.vector.tensor_scalar(
                        out=rk[:], in0=iwc_t[:],
                        scalar1=offr_t[:, c0 + k : c0 + k + 1],
                        scalar2=tw_t[:, k : k + 1],
                        op0=Op.is_equal, op1=Op.mult,
                    )
                    lhs = akh[:, k * P : (k + 1) * P]
                    for lo, hi in SEGS[k]:
                        nc.tensor.matmul(
                            out=psd[:, lo:hi],
                            lhsT=lhs,
                            rhs=rk[:, lo - BB[k] : hi - BB[k]],
                            start=True, stop=True,
                        )
                    nc.tensor.matmul(
                        out=psd[:, W + k * MAXMEM : W + (k + 1) * MAXMEM],
                        lhsT=lhs,
                        rhs=rk[:, FIXC0 : FIXC0 + MAXMEM],
                        start=True, stop=True,
                    )
                # ---- fixup class values + per-bank dense evacuation ----
                fixv = fixv_tp.tile([P, 1], F16, tag="fixv")
                nc.vector.tensor_reduce(
                    out=fixv[:], in_=psd[:, W : W + NB * MAXMEM],
                    axis=mybir.AxisListType.X, op=Op.max,
                )
                dense = dense_tp.tile([P, W], F16, tag="dense")
                for bk in range(4):
                    lo = bk * PSUM_BANK
                    hi = min((bk + 1) * PSUM_BANK, W)
                    nc.scalar.copy(out=dense[:, lo:hi], in_=psd[:, lo:hi])
                # ---- row writeback + fixup scatter (same SWDGE queue: ordered) ----
                nc.gpsimd.dma_start(
                    out=outs[r][:].rearrange("(p f) -> p f", f=W),
                    in_=dense[:],
                )
                nc.gpsimd.indirect_dma_start(
                    out=outs[r][:].unsqueeze(1),
                    out_offset=IndirectOffsetOnAxis(ap=fg_t[:, r : r + 1], axis=0),
                    in_=fixv[:, 0:1],
                    in_offset=None,
                    bounds_check=V - 1,
                    oob_is_err=False,
                )

    _split_excess_waits(nc)
    return nc


_prog_cache = {}


def _get_program():
    if "nc" not in _prog_cache:
        _prog_cache["nc"] = _build_program()
    return _prog_cache["nc"]


_BAND_OF = np.searchsorted(np.asarray(BB[1:]), np.arange(W), side="right")


def _make_in_maps(hidden_state, input_ids, w_sparse, b_sparse):
    hs = np.asarray(hidden_state, dtype=np.float32).reshape(B, L, H)
    ids_all = np.asarray(input_ids).astype(np.int64).reshape(B, L)
    w = np.asarray(w_sparse, dtype=np.float32).reshape(H)
    bval = float(np.asarray(b_sparse, dtype=np.float32).reshape(-1)[0])

    # constants shared by all cores
    wrep = np.ascontiguousarray(
        np.repeat(w.astype(np.float16).reshape(NB, P).T[:, :, None], P, axis=2)
        .reshape(P, NB * P)
    )
    ipt = np.broadcast_to(
        np.tile(np.arange(P, dtype=np.float16), NB), (P, NB * P)
    ).copy()
    iwc_row = np.full(RKW, -5.0, np.float32)
    iwc_row[0:FIXC0] = np.arange(FIXC0, dtype=np.float32)
    iwc_row[FIXC0:] = MEMMARK + np.arange(MAXMEM, dtype=np.float32)
    iwc = np.broadcast_to(iwc_row, (P, RKW)).copy()
    bcol = np.full((P, 1), bval, np.float32)

    in_maps = []
    for c in range(NCORES):
        ids = ids_all[c * BS : (c + 1) * BS]
        hsc = hs[c * BS : (c + 1) * BS].reshape(BS * L, H)
        route = np.full((P, BS * NB), -1.0, np.float16)
        offrel = np.full((P, BS * NB), -1.0, np.float32)
        fixgid = np.full((P, BS), V, np.int32)
        perm = np.full((BS * L,), -1, np.int64)

        for r in range(BS):
            row = ids[r]
            vals, counts = np.unique(row, return_counts=True)
            cnt = dict(zip(vals.tolist(), counts.tolist()))
            slots = [0] * NB
            nclass = 0
            classmem = {}
            fixup_tokens = []
            for l in range(L):
                v = int(row[l])
                if v < 4:
                    continue
                p, f = divmod(v, W)
                k = int(_BAND_OF[f])
                if cnt[v] == 1 and slots[k] < P:
                    s = slots[k]
                    slots[k] += 1
                    perm[r * L + k * P + s] = r * L + l
                    route[s, r * NB + k] = p
                    offrel[s, r * NB + k] = f - BB[k]
                else:
                    fixup_tokens.append((l, v))
            kf = 0
            memcnt = {}
            for l, v in fixup_tokens:
                if v in classmem:
                    q = classmem[v]
                else:
                    q = nclass
                    nclass += 1
                    assert nclass <= MAXCLS, f"too many fixup classes: {nclass}"
                    fixgid[q, r] = v
                    classmem[v] = q
                while slots[kf] >= P:
                    kf += 1
                m = memcnt.get((q, kf), 0)
                assert m < MAXMEM, "fixup class larger than MAXMEM in one chunk"
                memcnt[(q, kf)] = m + 1
                s = slots[kf]
                slots[kf] += 1
                perm[r * L + kf * P + s] = r * L + l
                route[s, r * NB + kf] = q
                offrel[s, r * NB + kf] = MEMMARK + m

        tmp = np.zeros((BS * L, H), np.float16)
        valid = perm >= 0
        tmp[valid] = hsc[perm[valid]].astype(np.float16)
        # xt2[r*128 + p, b*1024 + t] = hidden_f16[token(r, t), b*128 + p]
        xt2 = np.ascontiguousarray(
            tmp.reshape(BS, L, NB, P).transpose(0, 3, 2, 1).reshape(BS * P, NB * L)
        )
        in_maps.append(
            {
                "xt2": xt2,
                "wrep": wrep,
                "ipt": ipt,
                "iwc": iwc,
                "bcol": bcol,
                "route": route,
                "offrel": offrel,
                "fixgid": fixgid,
            }
        )
    return in_maps


def kernel(hidden_state, input_ids, w_sparse, b_sparse, _trace=False):
    nc = _get_program()
    in_maps = _make_in_maps(hidden_state, input_ids, w_sparse, b_sparse)
    res = run_bass_kernel_spmd(nc, in_maps, list(range(NCORES)), trace=_trace)
    out = np.empty((B, V), np.float32)
    for c in range(NCORES):
        for r in range(BS):
            out[c * BS + r] = np.asarray(res.results[c][f"out{r}"])[:V].astype(
                np.float32
            )
    if _trace:
        kernel.last_exec_time_ns = res.exec_time_ns
        kernel.last_results = res
    return out


# revision 6
# speedup vs baseline: 1.0676x; 1.0676x over previous
"""BGE-M3 sparse-embedding head (matvec + relu + scatter-max into (B, V))
as a Bass/Tile kernel on 8 Trainium2 NeuronCores.

Sharding: data-parallel over batch; each core computes 4 of 32 rows.

Per core / per row (1024 tokens):
  1. matvec on the PE: host uploads hidden TRANSPOSED ([h, token-slot] fp16);
     lhsT = w-block replicated across 128 columns, rhs = x^T -> every psum
     partition holds the full tw vector. Row 0 is copied to SBUF (Act) and
     round-tripped through a DRAM scratch so a strided DMA gather lands tw in
     token-major [128, 8] layout.
  2. Each token is host-assigned to (band-chunk k, slot s): chunk k holds only
     tokens whose dense column f = v % 1954 lies in fixed band k (~245 cols).
     DVE builds, per chunk, rk[slot, 0:246|246:262] = onehot(off)*tw covering
     both the band columns and 16 fixup member slots, plus one fused route
     one-hot akh[slot, k, p] (p = v // 1954, or fixup class q, or -1).
  3. PE assembles the dense [128, 1954] row in PSUM with one ~245-col matmul
     per band (disjoint -> start=stop=True, no zeroing pass) + one 16-col
     fixup matmul per chunk sharing the same lhsT. Act casts PSUM -> fp16.
  4. Duplicate ids / band-overflow tokens go to <=128 fixup classes: members
     accumulate into psum cols [1954, 1970), DVE reduce_max -> per-class value,
     scattered by ONE 128-index indirect DMA after the dense row write (both
     on the gpsimd SWDGE queue, so ordering is guaranteed by the queue).
Output is written fp16 (250112-padded) and upcast to f32 on the host.
"""

import numpy as np

import concourse.bass as bass
import concourse.mybir as mybir
import concourse.tile as tile
from concourse.bass import IndirectOffsetOnAxis
from concourse.bass_utils import run_bass_kernel_spmd

V = 250002
NCORES = 8
B, L, H = 32, 1024, 1024
BS = B // NCORES            # batch rows per core (4)
P = 128
W = 1954                    # dense row width per partition (128*1954 >= V)
VPAD = P * W                # 250112
NB = 8                      # h-blocks / band-chunks per row
MAXCLS = P                  # fixup classes per row
MAXMEM = 8                  # member slots per (fixup class, chunk)
FIXC0 = 246                 # rk column where fixup member slots start
MEMMARK = 2000.0            # iwc value marking member slot m: MEMMARK + m
RKW = FIXC0 + MAXMEM        # 254
F32 = mybir.dt.float32
F16 = mybir.dt.float16
I32 = mybir.dt.int32

# fixed band boundaries over [0, W)
BB = [round(W * k / NB) for k in range(NB + 1)]   # [0,244,489,733,977,1221,1466,1710,1954]
PSUM_BANK = 512  # f32 elements per PSUM bank; matmul out must not cross banks


def _band_segments():
    """Per band k: list of (lo, hi) psum column ranges split at bank bounds."""
    segs = []
    for k in range(NB):
        lo, hi = BB[k], BB[k + 1]
        cuts = [c for c in range(PSUM_BANK, 4 * PSUM_BANK, PSUM_BANK) if lo < c < hi]
        pts = [lo] + cuts + [hi]
        segs.append([(pts[i], pts[i + 1]) for i in range(len(pts) - 1)])
    return segs


SEGS = _band_segments()

_MAX_WAITS = 1


def _split_excess_waits(nc, cap=_MAX_WAITS):
    """walrus's gen3 codegen rejects >1 sync-wait per instruction; move the
    excess onto NoOps inserted just before (same engine => order kept)."""
    n = 0
    for func in nc.m.functions:
        for bb in func.blocks:
            newlist = []
            for ins in bb.instructions:
                si = getattr(ins, "sync_info", None)
                if si is not None and si.on_wait and len(si.on_wait) > cap:
                    waits = list(si.on_wait)
                    extra, keep = waits[:-cap], waits[-cap:]
                    while extra:
                        chunk, extra = extra[:cap], extra[cap:]
                        nop = mybir.InstNoOp(
                            name=f"{ins.name}-wsplit-{n}", ins=[], outs=[]
                        )
                        nop.engine = ins.engine
                        nop.sync_info = mybir.SyncInfo(on_wait=chunk, on_update=[])
                        newlist.append(nop)
                        n += 1
                    ins.sync_info = mybir.SyncInfo(
                        on_wait=keep, on_update=list(si.on_update)
                    )
                newlist.append(ins)
            bb.instructions = newlist
    return n


def _build_program():
    nc = bass.Bass()
    Op = mybir.AluOpType

    xt2 = nc.declare_dram_parameter("xt2", [BS * P, NB * L], F16, isOutput=False)
    wrep = nc.declare_dram_parameter("wrep", [P, NB * P], F16, isOutput=False)
    ipt = nc.declare_dram_parameter("ipt", [P, NB * P], F16, isOutput=False)
    iwc = nc.declare_dram_parameter("iwc", [P, RKW], F32, isOutput=False)
    bcol = nc.declare_dram_parameter("bcol", [P, 1], F32, isOutput=False)
    route = nc.declare_dram_parameter("route", [P, BS * NB], F16, isOutput=False)
    offrel = nc.declare_dram_parameter("offrel", [P, BS * NB], F32, isOutput=False)
    fixgid = nc.declare_dram_parameter("fixgid", [P, BS], I32, isOutput=False)
    outs = [
        nc.declare_dram_parameter(f"out{r}", [VPAD], F16, isOutput=True)
        for r in range(BS)
    ]
    twd = nc.dram_tensor("twd", [BS, L], F16, kind="Internal")

    with tile.TileContext(nc) as tc:
        with (
            tc.tile_pool(name="pers", bufs=1) as pers,
            tc.tile_pool(name="xt", bufs=3) as xt_tp,
            tc.tile_pool(name="akh", bufs=2) as akh_tp,
            tc.tile_pool(name="rk", bufs=3) as rk_tp,
            tc.tile_pool(name="tw", bufs=2) as tw_tp,
            tc.tile_pool(name="twrow", bufs=2) as twrow_tp,
            tc.tile_pool(name="dense", bufs=2) as dense_tp,
            tc.tile_pool(name="fixv", bufs=2) as fixv_tp,
            tc.tile_pool(name="ptw", bufs=2, space="PSUM") as ptw_tp,
            tc.tile_pool(name="psd", bufs=1, space="PSUM") as psd_tp,
        ):
            # ---- wrep first (matvec(0) needs it), then prefetch 2 rows of input ----
            wrep_t = pers.tile([P, NB * P], F16, tag="wrep")
            nc.sync.dma_start(out=wrep_t[:], in_=wrep[:])

            xt_tiles = {}

            def emit_in(r):
                xt_t = xt_tp.tile([P, NB * L], F16, tag="xt")
                nc.sync.dma_start(
                    out=xt_t[:, 0 : 4 * L],
                    in_=xt2[r * P : (r + 1) * P, 0 : 4 * L],
                )
                nc.scalar.dma_start(
                    out=xt_t[:, 4 * L : 8 * L],
                    in_=xt2[r * P : (r + 1) * P, 4 * L : 8 * L],
                )
                xt_tiles[r] = xt_t

            emit_in(0)
            emit_in(1)

            # ---- remaining constants (needed ~10us in, after the prefetches) ----
            ipt_t = pers.tile([P, NB * P], F16, tag="ipt")
            nc.sync.dma_start(out=ipt_t[:], in_=ipt[:])
            iwc_t = pers.tile([P, RKW], F32, tag="iwc")
            nc.sync.dma_start(out=iwc_t[:], in_=iwc[:])
            bcol_t = pers.tile([P, 1], F32, tag="bcol")
            nc.sync.dma_start(out=bcol_t[:], in_=bcol[:])
            route_t = pers.tile([P, BS * NB], F16, tag="route")
            nc.sync.dma_start(out=route_t[:], in_=route[:])
            offr_t = pers.tile([P, BS * NB], F32, tag="offr")
            nc.sync.dma_start(out=offr_t[:], in_=offrel[:])
            fg_t = pers.tile([P, BS], I32, tag="fg")
            nc.sync.dma_start(out=fg_t[:], in_=fixgid[:])

            ptw_tiles = {}

            def emit_matvec(r):
                ptw = ptw_tp.tile([P, L], F32, tag="ptw")
                xt_t = xt_tiles[r]
                for b in range(NB):
                    for hh in range(2):
                        nc.tensor.matmul(
                            out=ptw[:, hh * 512 : (hh + 1) * 512],
                            lhsT=wrep_t[:, b * P : (b + 1) * P],
                            rhs=xt_t[:, b * L + hh * 512 : b * L + (hh + 1) * 512],
                            start=(b == 0),
                            stop=(b == NB - 1),
                        )
                ptw_tiles[r] = ptw

            emit_matvec(0)

            for r in range(BS):
                c0 = r * NB
                if r + 2 < BS:
                    emit_in(r + 2)
                if r + 1 < BS:
                    emit_matvec(r + 1)  # PE fills the tw round-trip latency
                ptw = ptw_tiles.pop(r)
                xt_tiles.pop(r)
                # ---- tw extraction: psum row0 -> SBUF (transposed) -> DRAM -> [128, 8] ----
                twrow = twrow_tp.tile([1, L], F16, tag="twrow")
                nc.scalar.copy(
                    out=twrow[:].rearrange("o (p j) -> o j p", j=NB),
                    in_=ptw[0:1, :].rearrange("o (j p) -> o j p", p=P),
                )
                nc.gpsimd.dma_start(out=twd[r : r + 1, :], in_=twrow[:])
                twg = tw_tp.tile([P, NB], F16, tag="twg")
                nc.gpsimd.dma_start(
                    out=twg[:].rearrange("p (j o) -> p j o", o=1),
                    in_=twd[r : r + 1, :].rearrange("o (p j) -> p j o", p=P),
                )
                # relu(tw + b)
                tw_t = tw_tp.tile([P, NB], F32, tag="tw")
                nc.vector.tensor_scalar(
                    out=tw_t[:], in0=twg[:],
                    scalar1=bcol_t[:, 0:1], scalar2=0.0,
                    op0=Op.add, op1=Op.max,
                )
                # ---- route one-hot for all 8 chunks in one op ----
                akh = akh_tp.tile([P, NB * P], F16, tag="akh")
                nc.vector.tensor_tensor(
                    out=akh[:].rearrange("p (k q) -> p k q", q=P),
                    in0=ipt_t[:].rearrange("p (k q) -> p k q", q=P),
                    in1=route_t[:, c0 : c0 + NB].unsqueeze(2).broadcast_to((P, NB, P)),
                    op=Op.is_equal,
                )
                # ---- per-chunk rk + band/fixup matmuls ----
                psd = psd_tp.tile([P, 4 * PSUM_BANK], F32, tag="psd")
                for k in range(NB):
                    rk = rk_tp.tile([P, RKW], F16, tag="rk")
                    nc.vector.tensor_scalar(
                        out=rk[:], in0=iwc_t[:],
                        scalar1=offr_t[:, c0 + k : c0 + k + 1],
                        scalar2=tw_t[:, k : k + 1],
                        op0=Op.is_equal, op1=Op.mult,
                    )
                    lhs = akh[:, k * P : (k + 1) * P]
                    for lo, hi in SEGS[k]:
                        nc.tensor.matmul(
                            out=psd[:, lo:hi],
                            lhsT=lhs,
                            rhs=rk[:, lo - BB[k] : hi - BB[k]],
                            start=True, stop=True,
                        )
                    nc.tensor.matmul(
                        out=psd[:, W + k * MAXMEM : W + (k + 1) * MAXMEM],
                        lhsT=lhs,
                        rhs=rk[:, FIXC0 : FIXC0 + MAXMEM],
                        start=True, stop=True,
                    )
                # ---- fixup class values + per-bank dense evacuation ----
                fixv = fixv_tp.tile([P, 1], F16, tag="fixv")
                nc.vector.tensor_reduce(
                    out=fixv[:], in_=psd[:, W : W + NB * MAXMEM],
                    axis=mybir.AxisListType.X, op=Op.max,
                )
                dense = dense_tp.tile([P, W], F16, tag="dense")
                nc.scalar.copy(out=dense[:], in_=psd[:, 0:W])
                # ---- row writeback + fixup scatter (same SWDGE queue: ordered) ----
                nc.gpsimd.dma_start(
                    out=outs[r][:].rearrange("(p f) -> p f", f=W),
                    in_=dense[:],
                )
                nc.gpsimd.indirect_dma_start(
                    out=outs[r][:].unsqueeze(1),
                    out_offset=IndirectOffsetOnAxis(ap=fg_t[:, r : r + 1], axis=0),
                    in_=fixv[:, 0:1],
                    in_offset=None,
                    bounds_check=V - 1,
                    oob_is_err=False,
                )

    _split_excess_waits(nc)
    return nc


_prog_cache = {}


def _get_program():
    if "nc" not in _prog_cache:
        _prog_cache["nc"] = _build_program()
    return _prog_cache["nc"]


_BAND_OF = np.searchsorted(np.asarray(BB[1:]), np.arange(W), side="right")


def _make_in_maps(hidden_state, input_ids, w_sparse, b_sparse):
    hs = np.asarray(hidden_state, dtype=np.float32).reshape(B, L, H)
    ids_all = np.asarray(input_ids).astype(np.int64).reshape(B, L)
    w = np.asarray(w_sparse, dtype=np.float32).reshape(H)
    bval = float(np.asarray(b_sparse, dtype=np.float32).reshape(-1)[0])

    # constants shared by all cores
    wrep = np.ascontiguousarray(
        np.repeat(w.astype(np.float16).reshape(NB, P).T[:, :, None], P, axis=2)
        .reshape(P, NB * P)
    )
    ipt = np.broadcast_to(
        np.tile(np.arange(P, dtype=np.float16), NB), (P, NB * P)
    ).copy()
    iwc_row = np.full(RKW, -5.0, np.float32)
    iwc_row[0:FIXC0] = np.arange(FIXC0, dtype=np.float32)
    iwc_row[FIXC0:] = MEMMARK + np.arange(MAXMEM, dtype=np.float32)
    iwc = np.broadcast_to(iwc_row, (P, RKW)).copy()
    bcol = np.full((P, 1), bval, np.float32)

    in_maps = []
    for c in range(NCORES):
        ids = ids_all[c * BS : (c + 1) * BS]
        hsc = hs[c * BS : (c + 1) * BS].reshape(BS * L, H)
        route = np.full((P, BS * NB), -1.0, np.float16)
        offrel = np.full((P, BS * NB), -1.0, np.float32)
        fixgid = np.full((P, BS), V, np.int32)
        perm = np.full((BS * L,), -1, np.int64)

        for r in range(BS):
            row = ids[r]
            vals, counts = np.unique(row, return_counts=True)
            cnt = dict(zip(vals.tolist(), counts.tolist()))
            slots = [0] * NB
            nclass = 0
            classmem = {}
            fixup_tokens = []
            for l in range(L):
                v = int(row[l])
                if v < 4:
                    continue
                p, f = divmod(v, W)
                k = int(_BAND_OF[f])
                if cnt[v] == 1 and slots[k] < P:
                    s = slots[k]
                    slots[k] += 1
                    perm[r * L + k * P + s] = r * L + l
                    route[s, r * NB + k] = p
                    offrel[s, r * NB + k] = f - BB[k]
                else:
                    fixup_tokens.append((l, v))
            kf = 0
            memcnt = {}
            for l, v in fixup_tokens:
                if v in classmem:
                    q = classmem[v]
                else:
                    q = nclass
                    nclass += 1
                    assert nclass <= MAXCLS, f"too many fixup classes: {nclass}"
                    fixgid[q, r] = v
                    classmem[v] = q
                while slots[kf] >= P:
                    kf += 1
                m = memcnt.get((q, kf), 0)
                assert m < MAXMEM, "fixup class larger than MAXMEM in one chunk"
                memcnt[(q, kf)] = m + 1
                s = slots[kf]
                slots[kf] += 1
                perm[r * L + kf * P + s] = r * L + l
                route[s, r * NB + kf] = q
                offrel[s, r * NB + kf] = MEMMARK + m

        tmp = np.zeros((BS * L, H), np.float16)
        valid = perm >= 0
        tmp[valid] = hsc[perm[valid]].astype(np.float16)
        # xt2[r*128 + p, b*1024 + t] = hidden_f16[token(r, t), b*128 + p]
        xt2 = np.ascontiguousarray(
            tmp.reshape(BS, L, NB, P).transpose(0, 3, 2, 1).reshape(BS * P, NB * L)
        )
        in_maps.append(
            {
                "xt2": xt2,
                "wrep": wrep,
                "ipt": ipt,
                "iwc": iwc,
                "bcol": bcol,
                "route": route,
                "offrel": offrel,
                "fixgid": fixgid,
            }
        )
    return in_maps


def kernel(hidden_state, input_ids, w_sparse, b_sparse, _trace=False):
    nc = _get_program()
    in_maps = _make_in_maps(hidden_state, input_ids, w_sparse, b_sparse)
    res = run_bass_kernel_spmd(nc, in_maps, list(range(NCORES)), trace=_trace)
    out = np.empty((B, V), np.float32)
    for c in range(NCORES):
        for r in range(BS):
            out[c * BS + r] = np.asarray(res.results[c][f"out{r}"])[:V].astype(
                np.float32
            )
    if _trace:
        kernel.last_exec_time_ns = res.exec_time_ns
        kernel.last_results = res
    return out


# revision 7
# speedup vs baseline: 1.1426x; 1.0703x over previous
"""BGE-M3 sparse-embedding head (matvec + relu + scatter-max into (B, V))
as a Bass/Tile kernel on 8 Trainium2 NeuronCores.

Sharding: data-parallel over batch; each core computes 4 of 32 rows.

Per core / per row (1024 tokens):
  1. matvec on the PE: host uploads hidden TRANSPOSED ([h, token-slot] fp16);
     lhsT = w-block replicated across 128 columns, rhs = x^T -> every psum
     partition holds the full tw vector. Row 0 is copied to SBUF (Act) and
     round-tripped through a DRAM scratch so a strided DMA gather lands tw in
     token-major [128, 8] layout.
  2. Each token is host-assigned to (band-chunk k, slot s): chunk k holds only
     tokens whose dense column f = v % 1954 lies in fixed band k (~245 cols).
     DVE builds, per chunk, rk[slot, 0:246|246:262] = onehot(off)*tw covering
     both the band columns and 16 fixup member slots, plus one fused route
     one-hot akh[slot, k, p] (p = v // 1954, or fixup class q, or -1).
  3. PE assembles the dense [128, 1954] row in PSUM with one ~245-col matmul
     per band (disjoint -> start=stop=True, no zeroing pass) + one 16-col
     fixup matmul per chunk sharing the same lhsT. Act casts PSUM -> fp16.
  4. Duplicate ids / band-overflow tokens go to <=128 fixup classes: members
     accumulate into psum cols [1954, 1970), DVE reduce_max -> per-class value,
     scattered by ONE 128-index indirect DMA after the dense row write (both
     on the gpsimd SWDGE queue, so ordering is guaranteed by the queue).
Output is written fp16 (250112-padded) and upcast to f32 on the host.
"""

import numpy as np

import concourse.bass as bass
import concourse.mybir as mybir
import concourse.tile as tile
from concourse.bass import IndirectOffsetOnAxis
from concourse.bass_utils import run_bass_kernel_spmd

V = 250002
NCORES = 8
B, L, H = 32, 1024, 1024
BS = B // NCORES            # batch rows per core (4)
P = 128
W = 1954                    # dense row width per partition (128*1954 >= V)
VPAD = P * W                # 250112
NB = 8                      # h-blocks / band-chunks per row
MAXCLS = P                  # fixup classes per row
MAXMEM = 8                  # member slots per (fixup class, chunk)
FIXC0 = 246                 # rk column where fixup member slots start
MEMMARK = 2000.0            # iwc value marking member slot m: MEMMARK + m
RKW = FIXC0 + MAXMEM        # 254
F32 = mybir.dt.float32
F16 = mybir.dt.float16
I32 = mybir.dt.int32

# fixed band boundaries over [0, W)
BB = [round(W * k / NB) for k in range(NB + 1)]   # [0,244,489,733,977,1221,1466,1710,1954]
PSUM_BANK = 512  # f32 elements per PSUM bank; matmul out must not cross banks


def _band_segments():
    """Per band k: list of (lo, hi) psum column ranges split at bank bounds."""
    segs = []
    for k in range(NB):
        lo, hi = BB[k], BB[k + 1]
        cuts = [c for c in range(PSUM_BANK, 4 * PSUM_BANK, PSUM_BANK) if lo < c < hi]
        pts = [lo] + cuts + [hi]
        segs.append([(pts[i], pts[i + 1]) for i in range(len(pts) - 1)])
    return segs


SEGS = _band_segments()

_MAX_WAITS = 1


def _split_excess_waits(nc, cap=_MAX_WAITS):
    """walrus's gen3 codegen rejects >1 sync-wait per instruction; move the
    excess onto NoOps inserted just before (same engine => order kept)."""
    n = 0
    for func in nc.m.functions:
        for bb in func.blocks:
            newlist = []
            for ins in bb.instructions:
                si = getattr(ins, "sync_info", None)
                if si is not None and si.on_wait and len(si.on_wait) > cap:
                    waits = list(si.on_wait)
                    extra, keep = waits[:-cap], waits[-cap:]
                    while extra:
                        chunk, extra = extra[:cap], extra[cap:]
                        nop = mybir.InstNoOp(
                            name=f"{ins.name}-wsplit-{n}", ins=[], outs=[]
                        )
                        nop.engine = ins.engine
                        nop.sync_info = mybir.SyncInfo(on_wait=chunk, on_update=[])
                        newlist.append(nop)
                        n += 1
                    ins.sync_info = mybir.SyncInfo(
                        on_wait=keep, on_update=list(si.on_update)
                    )
                newlist.append(ins)
            bb.instructions = newlist
    return n


def _build_program():
    nc = bass.Bass()
    Op = mybir.AluOpType

    xt2 = nc.declare_dram_parameter("xt2", [BS * P, NB * L], F16, isOutput=False)
    wrep = nc.declare_dram_parameter("wrep", [P, NB * P], F16, isOutput=False)
    ipt = nc.declare_dram_parameter("ipt", [P, NB * P], F16, isOutput=False)
    iwc = nc.declare_dram_parameter("iwc", [P, RKW], F32, isOutput=False)
    bcol = nc.declare_dram_parameter("bcol", [P, 1], F32, isOutput=False)
    route = nc.declare_dram_parameter("route", [P, BS * NB], F16, isOutput=False)
    offrel = nc.declare_dram_parameter("offrel", [P, BS * NB], F32, isOutput=False)
    fixgid = nc.declare_dram_parameter("fixgid", [P, BS], I32, isOutput=False)
    outs = [
        nc.declare_dram_parameter(f"out{r}", [VPAD], F16, isOutput=True)
        for r in range(BS)
    ]

    with tile.TileContext(nc) as tc:
        with (
            tc.tile_pool(name="pers", bufs=1) as pers,
            tc.tile_pool(name="xt", bufs=3) as xt_tp,
            tc.tile_pool(name="akh", bufs=2) as akh_tp,
            tc.tile_pool(name="rk", bufs=3) as rk_tp,
            tc.tile_pool(name="tw", bufs=2) as tw_tp,
            tc.tile_pool(name="twrow", bufs=2) as twrow_tp,
            tc.tile_pool(name="dense", bufs=2) as dense_tp,
            tc.tile_pool(name="fixv", bufs=2) as fixv_tp,
            tc.tile_pool(name="ptw", bufs=2, space="PSUM") as ptw_tp,
            tc.tile_pool(name="psd", bufs=1, space="PSUM") as psd_tp,
        ):
            # ---- wrep first (matvec(0) needs it), then prefetch 2 rows of input ----
            wrep_t = pers.tile([P, NB * P], F16, tag="wrep")
            nc.sync.dma_start(out=wrep_t[:], in_=wrep[:])

            xt_tiles = {}

            def emit_in(r):
                xt_t = xt_tp.tile([P, NB * L], F16, tag="xt")
                nc.sync.dma_start(
                    out=xt_t[:, 0 : 4 * L],
                    in_=xt2[r * P : (r + 1) * P, 0 : 4 * L],
                )
                nc.scalar.dma_start(
                    out=xt_t[:, 4 * L : 8 * L],
                    in_=xt2[r * P : (r + 1) * P, 4 * L : 8 * L],
                )
                xt_tiles[r] = xt_t

            emit_in(0)
            emit_in(1)

            # ---- remaining constants (needed ~10us in, after the prefetches) ----
            ipt_t = pers.tile([P, NB * P], F16, tag="ipt")
            nc.sync.dma_start(out=ipt_t[:], in_=ipt[:])
            iwc_t = pers.tile([P, RKW], F32, tag="iwc")
            nc.sync.dma_start(out=iwc_t[:], in_=iwc[:])
            bcol_t = pers.tile([P, 1], F32, tag="bcol")
            nc.sync.dma_start(out=bcol_t[:], in_=bcol[:])
            route_t = pers.tile([P, BS * NB], F16, tag="route")
            nc.sync.dma_start(out=route_t[:], in_=route[:])
            offr_t = pers.tile([P, BS * NB], F32, tag="offr")
            nc.sync.dma_start(out=offr_t[:], in_=offrel[:])
            fg_t = pers.tile([P, BS], I32, tag="fg")
            nc.sync.dma_start(out=fg_t[:], in_=fixgid[:])

            ones1_t = pers.tile([1, 1], F32, tag="ones1")
            nc.vector.memset(ones1_t[:], 1.0)

            ptw_tiles = {}

            def emit_matvec(r):
                ptw = ptw_tp.tile([P, L], F32, tag="ptw")
                xt_t = xt_tiles[r]
                for b in range(NB):
                    for hh in range(2):
                        nc.tensor.matmul(
                            out=ptw[:, hh * 512 : (hh + 1) * 512],
                            lhsT=wrep_t[:, b * P : (b + 1) * P],
                            rhs=xt_t[:, b * L + hh * 512 : b * L + (hh + 1) * 512],
                            start=(b == 0),
                            stop=(b == NB - 1),
                        )
                ptw_tiles[r] = ptw

            emit_matvec(0)

            for r in range(BS):
                c0 = r * NB
                if r + 2 < BS:
                    emit_in(r + 2)
                if r + 1 < BS:
                    emit_matvec(r + 1)  # PE fills the tw round-trip latency
                ptw = ptw_tiles.pop(r)
                xt_tiles.pop(r)
                psd = psd_tp.tile([P, 4 * PSUM_BANK], F32, tag="psd")
                # ---- tw extraction: psum row0 -> SBUF, 8 PE transposes -> [128, 8] ----
                twrow = twrow_tp.tile([1, L], F32, tag="twrow")
                nc.scalar.copy(out=twrow[:], in_=ptw[0:1, :])
                TW0 = 4 * PSUM_BANK - NB  # tw column block in psd spare space
                for j in range(NB):
                    nc.tensor.transpose(
                        out=psd[:, TW0 + j : TW0 + j + 1],
                        in_=twrow[0:1, j * P : (j + 1) * P],
                        identity=ones1_t[:],
                    )
                # relu(tw + b)
                tw_t = tw_tp.tile([P, NB], F32, tag="tw")
                nc.vector.tensor_scalar(
                    out=tw_t[:], in0=psd[:, TW0 : TW0 + NB],
                    scalar1=bcol_t[:, 0:1], scalar2=0.0,
                    op0=Op.add, op1=Op.max,
                )
                # ---- route one-hot for all 8 chunks in one op ----
                akh = akh_tp.tile([P, NB * P], F16, tag="akh")
                nc.vector.tensor_tensor(
                    out=akh[:].rearrange("p (k q) -> p k q", q=P),
                    in0=ipt_t[:].rearrange("p (k q) -> p k q", q=P),
                    in1=route_t[:, c0 : c0 + NB].unsqueeze(2).broadcast_to((P, NB, P)),
                    op=Op.is_equal,
                )
                # ---- per-chunk rk + band/fixup matmuls ----
                for k in range(NB):
                    rk = rk_tp.tile([P, RKW], F16, tag="rk")
                    nc.vector.tensor_scalar(
                        out=rk[:], in0=iwc_t[:],
                        scalar1=offr_t[:, c0 + k : c0 + k + 1],
                        scalar2=tw_t[:, k : k + 1],
                        op0=Op.is_equal, op1=Op.mult,
                    )
                    lhs = akh[:, k * P : (k + 1) * P]
                    for lo, hi in SEGS[k]:
                        nc.tensor.matmul(
                            out=psd[:, lo:hi],
                            lhsT=lhs,
                            rhs=rk[:, lo - BB[k] : hi - BB[k]],
                            start=True, stop=True,
                        )
                    nc.tensor.matmul(
                        out=psd[:, W + k * MAXMEM : W + (k + 1) * MAXMEM],
                        lhsT=lhs,
                        rhs=rk[:, FIXC0 : FIXC0 + MAXMEM],
                        start=True, stop=True,
                    )
                # ---- fixup class values + per-bank dense evacuation ----
                fixv = fixv_tp.tile([P, 1], F16, tag="fixv")
                nc.vector.tensor_reduce(
                    out=fixv[:], in_=psd[:, W : W + NB * MAXMEM],
                    axis=mybir.AxisListType.X, op=Op.max,
                )
                dense = dense_tp.tile([P, W], F16, tag="dense")
                nc.scalar.copy(out=dense[:], in_=psd[:, 0:W])
                # ---- row writeback + fixup scatter (same SWDGE queue: ordered) ----
                nc.gpsimd.dma_start(
                    out=outs[r][:].rearrange("(p f) -> p f", f=W),
                    in_=dense[:],
                )
                nc.gpsimd.indirect_dma_start(
                    out=outs[r][:].unsqueeze(1),
                    out_offset=IndirectOffsetOnAxis(ap=fg_t[:, r : r + 1], axis=0),
                    in_=fixv[:, 0:1],
                    in_offset=None,
                    bounds_check=V - 1,
                    oob_is_err=False,
                )

    _split_excess_waits(nc)
    return nc


_prog_cache = {}


def _get_program():
    if "nc" not in _prog_cache:
        _prog_cache["nc"] = _build_program()
    return _prog_cache["nc"]


_BAND_OF = np.searchsorted(np.asarray(BB[1:]), np.arange(W), side="right")


def _make_in_maps(hidden_state, input_ids, w_sparse, b_sparse):
    hs = np.asarray(hidden_state, dtype=np.float32).reshape(B, L, H)
    ids_all = np.asarray(input_ids).astype(np.int64).reshape(B, L)
    w = np.asarray(w_sparse, dtype=np.float32).reshape(H)
    bval = float(np.asarray(b_sparse, dtype=np.float32).reshape(-1)[0])

    # constants shared by all cores
    wrep = np.ascontiguousarray(
        np.repeat(w.astype(np.float16).reshape(NB, P).T[:, :, None], P, axis=2)
        .reshape(P, NB * P)
    )
    ipt = np.broadcast_to(
        np.tile(np.arange(P, dtype=np.float16), NB), (P, NB * P)
    ).copy()
    iwc_row = np.full(RKW, -5.0, np.float32)
    iwc_row[0:FIXC0] = np.arange(FIXC0, dtype=np.float32)
    iwc_row[FIXC0:] = MEMMARK + np.arange(MAXMEM, dtype=np.float32)
    iwc = np.broadcast_to(iwc_row, (P, RKW)).copy()
    bcol = np.full((P, 1), bval, np.float32)

    in_maps = []
    for c in range(NCORES):
        ids = ids_all[c * BS : (c + 1) * BS]
        hsc = hs[c * BS : (c + 1) * BS].reshape(BS * L, H)
        route = np.full((P, BS * NB), -1.0, np.float16)
        offrel = np.full((P, BS * NB), -1.0, np.float32)
        fixgid = np.full((P, BS), V, np.int32)
        perm = np.full((BS * L,), -1, np.int64)

        for r in range(BS):
            row = ids[r]
            vals, counts = np.unique(row, return_counts=True)
            cnt = dict(zip(vals.tolist(), counts.tolist()))
            slots = [0] * NB
            nclass = 0
            classmem = {}
            fixup_tokens = []
            for l in range(L):
                v = int(row[l])
                if v < 4:
                    continue
                p, f = divmod(v, W)
                k = int(_BAND_OF[f])
                if cnt[v] == 1 and slots[k] < P:
                    s = slots[k]
                    slots[k] += 1
                    perm[r * L + k * P + s] = r * L + l
                    route[s, r * NB + k] = p
                    offrel[s, r * NB + k] = f - BB[k]
                else:
                    fixup_tokens.append((l, v))
            kf = 0
            memcnt = {}
            for l, v in fixup_tokens:
                if v in classmem:
                    q = classmem[v]
                else:
                    q = nclass
                    nclass += 1
                    assert nclass <= MAXCLS, f"too many fixup classes: {nclass}"
                    fixgid[q, r] = v
                    classmem[v] = q
                while slots[kf] >= P:
                    kf += 1
                m = memcnt.get((q, kf), 0)
                assert m < MAXMEM, "fixup class larger than MAXMEM in one chunk"
                memcnt[(q, kf)] = m + 1
                s = slots[kf]
                slots[kf] += 1
                perm[r * L + kf * P + s] = r * L + l
                route[s, r * NB + kf] = q
                offrel[s, r * NB + kf] = MEMMARK + m

        tmp = np.zeros((BS * L, H), np.float16)
        valid = perm >= 0
        tmp[valid] = hsc[perm[valid]].astype(np.float16)
        # xt2[r*128 + p, b*1024 + t] = hidden_f16[token(r, t), b*128 + p]
        xt2 = np.ascontiguousarray(
            tmp.reshape(BS, L, NB, P).transpose(0, 3, 2, 1).reshape(BS * P, NB * L)
        )
        in_maps.append(
            {
                "xt2": xt2,
                "wrep": wrep,
                "ipt": ipt,
                "iwc": iwc,
                "bcol": bcol,
                "route": route,
                "offrel": offrel,
                "fixgid": fixgid,
            }
        )
    return in_maps


def kernel(hidden_state, input_ids, w_sparse, b_sparse, _trace=False):
    nc = _get_program()
    in_maps = _make_in_maps(hidden_state, input_ids, w_sparse, b_sparse)
    res = run_bass_kernel_spmd(nc, in_maps, list(range(NCORES)), trace=_trace)
    out = np.empty((B, V), np.float32)
    for c in range(NCORES):
        for r in range(BS):
            out[c * BS + r] = np.asarray(res.results[c][f"out{r}"])[:V].astype(
                np.float32
            )
    if _trace:
        kernel.last_exec_time_ns = res.exec_time_ns
        kernel.last_results = res
    return out


# revision 8
# speedup vs baseline: 1.1655x; 1.0200x over previous
"""BGE-M3 sparse-embedding head (matvec + relu + scatter-max into (B, V))
as a Bass/Tile kernel on 8 Trainium2 NeuronCores.

Sharding: data-parallel over batch; each core computes 4 of 32 rows.

Per core / per row (1024 tokens):
  1. matvec on the PE: host uploads hidden TRANSPOSED ([h, token-slot] fp16);
     lhsT = w-block replicated across 128 columns, rhs = x^T -> every psum
     partition holds the full tw vector. Row 0 is copied to SBUF (Act) and
     round-tripped through a DRAM scratch so a strided DMA gather lands tw in
     token-major [128, 8] layout.
  2. Each token is host-assigned to (band-chunk k, slot s): chunk k holds only
     tokens whose dense column f = v % 1954 lies in fixed band k (~245 cols).
     DVE builds, per chunk, rk[slot, 0:246|246:262] = onehot(off)*tw covering
     both the band columns and 16 fixup member slots, plus one fused route
     one-hot akh[slot, k, p] (p = v // 1954, or fixup class q, or -1).
  3. PE assembles the dense [128, 1954] row in PSUM with one ~245-col matmul
     per band (disjoint -> start=stop=True, no zeroing pass) + one 16-col
     fixup matmul per chunk sharing the same lhsT. Act casts PSUM -> fp16.
  4. Duplicate ids / band-overflow tokens go to <=128 fixup classes: members
     accumulate into psum cols [1954, 1970), DVE reduce_max -> per-class value,
     scattered by ONE 128-index indirect DMA after the dense row write (both
     on the gpsimd SWDGE queue, so ordering is guaranteed by the queue).
Output is written fp16 (250112-padded) and upcast to f32 on the host.
"""

import numpy as np

import concourse.bass as bass
import concourse.mybir as mybir
import concourse.tile as tile
from concourse.bass import IndirectOffsetOnAxis
from concourse.bass_utils import run_bass_kernel_spmd

V = 250002
NCORES = 8
B, L, H = 32, 1024, 1024
BS = B // NCORES            # batch rows per core (4)
P = 128
W = 1954                    # dense row width per partition (128*1954 >= V)
VPAD = P * W                # 250112
NB = 8                      # h-blocks / band-chunks per row
MAXCLS = P                  # fixup classes per row
MAXMEM = 8                  # member slots per (fixup class, chunk)
FIXC0 = 246                 # rk column where fixup member slots start
MEMMARK = 2000.0            # iwc value marking member slot m: MEMMARK + m
RKW = FIXC0 + MAXMEM        # 254
F32 = mybir.dt.float32
F16 = mybir.dt.float16
I32 = mybir.dt.int32

# fixed band boundaries over [0, W)
BB = [round(W * k / NB) for k in range(NB + 1)]   # [0,244,489,733,977,1221,1466,1710,1954]
PSUM_BANK = 512  # f32 elements per PSUM bank; matmul out must not cross banks


def _band_segments():
    """Per band k: list of (lo, hi) psum column ranges split at bank bounds."""
    segs = []
    for k in range(NB):
        lo, hi = BB[k], BB[k + 1]
        cuts = [c for c in range(PSUM_BANK, 4 * PSUM_BANK, PSUM_BANK) if lo < c < hi]
        pts = [lo] + cuts + [hi]
        segs.append([(pts[i], pts[i + 1]) for i in range(len(pts) - 1)])
    return segs


SEGS = _band_segments()

_MAX_WAITS = 1


def _split_excess_waits(nc, cap=_MAX_WAITS):
    """walrus's gen3 codegen rejects >1 sync-wait per instruction; move the
    excess onto NoOps inserted just before (same engine => order kept)."""
    n = 0
    for func in nc.m.functions:
        for bb in func.blocks:
            newlist = []
            for ins in bb.instructions:
                si = getattr(ins, "sync_info", None)
                if si is not None and si.on_wait and len(si.on_wait) > cap:
                    waits = list(si.on_wait)
                    extra, keep = waits[:-cap], waits[-cap:]
                    while extra:
                        chunk, extra = extra[:cap], extra[cap:]
                        nop = mybir.InstNoOp(
                            name=f"{ins.name}-wsplit-{n}", ins=[], outs=[]
                        )
                        nop.engine = ins.engine
                        nop.sync_info = mybir.SyncInfo(on_wait=chunk, on_update=[])
                        newlist.append(nop)
                        n += 1
                    ins.sync_info = mybir.SyncInfo(
                        on_wait=keep, on_update=list(si.on_update)
                    )
                newlist.append(ins)
            bb.instructions = newlist
    return n


def _build_program():
    nc = bass.Bass()
    Op = mybir.AluOpType

    xt2 = nc.declare_dram_parameter("xt2", [BS * P, NB * L], F16, isOutput=False)
    wrep = nc.declare_dram_parameter("wrep", [P, NB * P], F16, isOutput=False)
    ipt = nc.declare_dram_parameter("ipt", [P, NB * P], F16, isOutput=False)
    iwc = nc.declare_dram_parameter("iwc", [P, RKW], F32, isOutput=False)
    bcol = nc.declare_dram_parameter("bcol", [P, 1], F32, isOutput=False)
    route = nc.declare_dram_parameter("route", [P, BS * NB], F16, isOutput=False)
    offrel = nc.declare_dram_parameter("offrel", [P, BS * NB], F32, isOutput=False)
    fixgid = nc.declare_dram_parameter("fixgid", [P, BS], I32, isOutput=False)
    outs = [
        nc.declare_dram_parameter(f"out{r}", [VPAD], F16, isOutput=True)
        for r in range(BS)
    ]

    with tile.TileContext(nc) as tc:
        with (
            tc.tile_pool(name="pers", bufs=1) as pers,
            tc.tile_pool(name="xt", bufs=4) as xt_tp,
            tc.tile_pool(name="akh", bufs=2) as akh_tp,
            tc.tile_pool(name="rk", bufs=3) as rk_tp,
            tc.tile_pool(name="tw", bufs=2) as tw_tp,
            tc.tile_pool(name="twrow", bufs=2) as twrow_tp,
            tc.tile_pool(name="dense", bufs=2) as dense_tp,
            tc.tile_pool(name="fixv", bufs=2) as fixv_tp,
            tc.tile_pool(name="ptw", bufs=2, space="PSUM") as ptw_tp,
            tc.tile_pool(name="psd", bufs=1, space="PSUM") as psd_tp,
        ):
            # ---- wrep first (matvec(0) needs it), then prefetch 2 rows of input ----
            wrep_t = pers.tile([P, NB * P], F16, tag="wrep")
            nc.sync.dma_start(out=wrep_t[:], in_=wrep[:])

            xt_tiles = {}

            def emit_in(r, fine=False):
                xt_t = xt_tp.tile([P, NB * L], F16, tag="xt")
                if fine:
                    for i, eng in enumerate((nc.sync, nc.scalar, nc.sync, nc.scalar)):
                        eng.dma_start(
                            out=xt_t[:, i * 2 * L : (i + 1) * 2 * L],
                            in_=xt2[r * P : (r + 1) * P, i * 2 * L : (i + 1) * 2 * L],
                        )
                else:
                    nc.sync.dma_start(
                        out=xt_t[:, 0 : 4 * L],
                        in_=xt2[r * P : (r + 1) * P, 0 : 4 * L],
                    )
                    nc.scalar.dma_start(
                        out=xt_t[:, 4 * L : 8 * L],
                        in_=xt2[r * P : (r + 1) * P, 4 * L : 8 * L],
                    )
                xt_tiles[r] = xt_t

            # ---- constants on the (idle) gpsimd queue ----
            ipt_t = pers.tile([P, NB * P], F16, tag="ipt")
            nc.gpsimd.dma_start(out=ipt_t[:], in_=ipt[:])
            iwc_t = pers.tile([P, RKW], F32, tag="iwc")
            nc.gpsimd.dma_start(out=iwc_t[:], in_=iwc[:])
            bcol_t = pers.tile([P, 1], F32, tag="bcol")
            nc.gpsimd.dma_start(out=bcol_t[:], in_=bcol[:])
            route_t = pers.tile([P, BS * NB], F16, tag="route")
            nc.gpsimd.dma_start(out=route_t[:], in_=route[:])
            offr_t = pers.tile([P, BS * NB], F32, tag="offr")
            nc.gpsimd.dma_start(out=offr_t[:], in_=offrel[:])
            fg_t = pers.tile([P, BS], I32, tag="fg")
            nc.gpsimd.dma_start(out=fg_t[:], in_=fixgid[:])

            emit_in(0, fine=True)
            emit_in(1)

            ones1_t = pers.tile([1, 1], F32, tag="ones1")
            nc.vector.memset(ones1_t[:], 1.0)

            ptw_tiles = {}

            def emit_matvec(r):
                ptw = ptw_tp.tile([P, L], F32, tag="ptw")
                xt_t = xt_tiles[r]
                for b in range(NB):
                    for hh in range(2):
                        nc.tensor.matmul(
                            out=ptw[:, hh * 512 : (hh + 1) * 512],
                            lhsT=wrep_t[:, b * P : (b + 1) * P],
                            rhs=xt_t[:, b * L + hh * 512 : b * L + (hh + 1) * 512],
                            start=(b == 0),
                            stop=(b == NB - 1),
                        )
                ptw_tiles[r] = ptw

            emit_matvec(0)

            for r in range(BS):
                c0 = r * NB
                if r + 2 < BS:
                    emit_in(r + 2)
                ptw = ptw_tiles.pop(r)
                xt_tiles.pop(r)
                psd = psd_tp.tile([P, 4 * PSUM_BANK], F32, tag="psd")
                # ---- tw extraction: psum row0 -> SBUF, 8 PE transposes -> [128, 8] ----
                twrow = twrow_tp.tile([1, L], F32, tag="twrow")
                nc.scalar.copy(out=twrow[:], in_=ptw[0:1, :])
                TW0 = 4 * PSUM_BANK - NB  # tw column block in psd spare space
                for j in range(NB):
                    nc.tensor.transpose(
                        out=psd[:, TW0 + j : TW0 + j + 1],
                        in_=twrow[0:1, j * P : (j + 1) * P],
                        identity=ones1_t[:],
                    )
                if r + 1 < BS:
                    emit_matvec(r + 1)  # PE fills the tw->rk dependency latency
                # relu(tw + b)
                tw_t = tw_tp.tile([P, NB], F32, tag="tw")
                nc.vector.tensor_scalar(
                    out=tw_t[:], in0=psd[:, TW0 : TW0 + NB],
                    scalar1=bcol_t[:, 0:1], scalar2=0.0,
                    op0=Op.add, op1=Op.max,
                )
                # ---- route one-hot for all 8 chunks in one op ----
                akh = akh_tp.tile([P, NB * P], F16, tag="akh")
                nc.vector.tensor_tensor(
                    out=akh[:].rearrange("p (k q) -> p k q", q=P),
                    in0=ipt_t[:].rearrange("p (k q) -> p k q", q=P),
                    in1=route_t[:, c0 : c0 + NB].unsqueeze(2).broadcast_to((P, NB, P)),
                    op=Op.is_equal,
                )
                # ---- per-chunk rk + band/fixup matmuls ----
                for k in range(NB):
                    rk = rk_tp.tile([P, RKW], F16, tag="rk")
                    nc.vector.tensor_scalar(
                        out=rk[:], in0=iwc_t[:],
                        scalar1=offr_t[:, c0 + k : c0 + k + 1],
                        scalar2=tw_t[:, k : k + 1],
                        op0=Op.is_equal, op1=Op.mult,
                    )
                    lhs = akh[:, k * P : (k + 1) * P]
                    for lo, hi in SEGS[k]:
                        nc.tensor.matmul(
                            out=psd[:, lo:hi],
                            lhsT=lhs,
                            rhs=rk[:, lo - BB[k] : hi - BB[k]],
                            start=True, stop=True,
                        )
                    nc.tensor.matmul(
                        out=psd[:, W + k * MAXMEM : W + (k + 1) * MAXMEM],
                        lhsT=lhs,
                        rhs=rk[:, FIXC0 : FIXC0 + MAXMEM],
                        start=True, stop=True,
                    )
                # ---- fixup class values + per-bank dense evacuation ----
                fixv = fixv_tp.tile([P, 1], F16, tag="fixv")
                nc.vector.tensor_reduce(
                    out=fixv[:], in_=psd[:, W : W + NB * MAXMEM],
                    axis=mybir.AxisListType.X, op=Op.max,
                )
                dense = dense_tp.tile([P, W], F16, tag="dense")
                nc.scalar.copy(out=dense[:], in_=psd[:, 0:W])
                # ---- row writeback + fixup scatter (same SWDGE queue: ordered) ----
                nc.gpsimd.dma_start(
                    out=outs[r][:].rearrange("(p f) -> p f", f=W),
                    in_=dense[:],
                )
                nc.gpsimd.indirect_dma_start(
                    out=outs[r][:].unsqueeze(1),
                    out_offset=IndirectOffsetOnAxis(ap=fg_t[:, r : r + 1], axis=0),
                    in_=fixv[:, 0:1],
                    in_offset=None,
                    bounds_check=V - 1,
                    oob_is_err=False,
                )

    _split_excess_waits(nc)
    return nc


_prog_cache = {}


def _get_program():
    if "nc" not in _prog_cache:
        _prog_cache["nc"] = _build_program()
    return _prog_cache["nc"]


_BAND_OF = np.searchsorted(np.asarray(BB[1:]), np.arange(W), side="right")


def _make_in_maps(hidden_state, input_ids, w_sparse, b_sparse):
    hs = np.asarray(hidden_state, dtype=np.float32).reshape(B, L, H)
    ids_all = np.asarray(input_ids).astype(np.int64).reshape(B, L)
    w = np.asarray(w_sparse, dtype=np.float32).reshape(H)
    bval = float(np.asarray(b_sparse, dtype=np.float32).reshape(-1)[0])

    # constants shared by all cores
    wrep = np.ascontiguousarray(
        np.repeat(w.astype(np.float16).reshape(NB, P).T[:, :, None], P, axis=2)
        .reshape(P, NB * P)
    )
    ipt = np.broadcast_to(
        np.tile(np.arange(P, dtype=np.float16), NB), (P, NB * P)
    ).copy()
    iwc_row = np.full(RKW, -5.0, np.float32)
    iwc_row[0:FIXC0] = np.arange(FIXC0, dtype=np.float32)
    iwc_row[FIXC0:] = MEMMARK + np.arange(MAXMEM, dtype=np.float32)
    iwc = np.broadcast_to(iwc_row, (P, RKW)).copy()
    bcol = np.full((P, 1), bval, np.float32)

    in_maps = []
    for c in range(NCORES):
        ids = ids_all[c * BS : (c + 1) * BS]
        hsc = hs[c * BS : (c + 1) * BS].reshape(BS * L, H)
        route = np.full((P, BS * NB), -1.0, np.float16)
        offrel = np.full((P, BS * NB), -1.0, np.float32)
        fixgid = np.full((P, BS), V, np.int32)
        perm = np.full((BS * L,), -1, np.int64)

        for r in range(BS):
            row = ids[r]
            vals, counts = np.unique(row, return_counts=True)
            cnt = dict(zip(vals.tolist(), counts.tolist()))
            slots = [0] * NB
            nclass = 0
            classmem = {}
            fixup_tokens = []
            for l in range(L):
                v = int(row[l])
                if v < 4:
                    continue
                p, f = divmod(v, W)
                k = int(_BAND_OF[f])
                if cnt[v] == 1 and slots[k] < P:
                    s = slots[k]
                    slots[k] += 1
                    perm[r * L + k * P + s] = r * L + l
                    route[s, r * NB + k] = p
                    offrel[s, r * NB + k] = f - BB[k]
                else:
                    fixup_tokens.append((l, v))
            kf = 0
            memcnt = {}
            for l, v in fixup_tokens:
                if v in classmem:
                    q = classmem[v]
                else:
                    q = nclass
                    nclass += 1
                    assert nclass <= MAXCLS, f"too many fixup classes: {nclass}"
                    fixgid[q, r] = v
                    classmem[v] = q
                while slots[kf] >= P:
                    kf += 1
                m = memcnt.get((q, kf), 0)
                assert m < MAXMEM, "fixup class larger than MAXMEM in one chunk"
                memcnt[(q, kf)] = m + 1
                s = slots[kf]
                slots[kf] += 1
                perm[r * L + kf * P + s] = r * L + l
                route[s, r * NB + kf] = q
                offrel[s, r * NB + kf] = MEMMARK + m

        tmp = np.zeros((BS * L, H), np.float16)
        valid = perm >= 0
        tmp[valid] = hsc[perm[valid]].astype(np.float16)
        # xt2[r*128 + p, b*1024 + t] = hidden_f16[token(r, t), b*128 + p]
        xt2 = np.ascontiguousarray(
            tmp.reshape(BS, L, NB, P).transpose(0, 3, 2, 1).reshape(BS * P, NB * L)
        )
        in_maps.append(
            {
                "xt2": xt2,
                "wrep": wrep,
                "ipt": ipt,
                "iwc": iwc,
                "bcol": bcol,
                "route": route,
                "offrel": offrel,
                "fixgid": fixgid,
            }
        )
    return in_maps


def kernel(hidden_state, input_ids, w_sparse, b_sparse, _trace=False):
    nc = _get_program()
    in_maps = _make_in_maps(hidden_state, input_ids, w_sparse, b_sparse)
    res = run_bass_kernel_spmd(nc, in_maps, list(range(NCORES)), trace=_trace)
    out = np.empty((B, V), np.float32)
    for c in range(NCORES):
        for r in range(BS):
            out[c * BS + r] = np.asarray(res.results[c][f"out{r}"])[:V].astype(
                np.float32
            )
    if _trace:
        kernel.last_exec_time_ns = res.exec_time_ns
        kernel.last_results = res
    return out
